# revision 62
# baseline (speedup 1.0000x reference)
"""MeanField CRF message-passing kernel for 8 Trainium2 NeuronCores.

Sharding: (B=2) x (H into 4 chunks of 128 rows) = 8 slabs, each with a
5-row halo on slab-interior edges (5 mean-field iterations x 1-row
stencil reach), so cores run fully independently (no collectives).

Per-core layouts (bf16 state for DVE 2x / PE 1-cycle-per-row modes):
  y-major : [x mod 128 -> partitions, (xblock, y, class) -> free]  (Y only)
  c-major : [x mod 128 -> partitions, (xblock, class, y) -> free]
  C-packed: [(y mod 6, class) -> 126 partitions, x -> free]
Math per iteration (equivalent-transformed from the reference):
  YC   = PE-transpose(Y)                 (bf16 PSUM; Y y-major so the
                                          transpose input is a flat AP)
  EC   = exp(-YC)                        (ACT reads PSUM, fuses evac)
  m    = ECslice^T @ LCB2 ; s = ECslice^T @ J6   (PE; fuses LC^T/8
                                          contraction with transpose back)
  r    = 1/s                             (DVE), folded into w2
  mxp/mxm = x+-1 partition-shifted m     (DMA, idle engine)
  w2_d = ew_d * shift_d(r)               (DVE TT, bf16 2x)
  t_d  = w2_d(bcast C) * shift_d(m)      (DVE TT, bf16 2x, c-major)
  Ypsum= I^T@u + sum_d I^T@t_d           (PE identity-matmul PSUM accum
                                          -> replaces all DVE adds)
  Y    = evac(Ypsum)                     (ACT, scatter to y-major bf16)
Final cost = Y after iteration 5, stored bf16 (~2.4e-3 rel err, gate 2e-2).

Head-latency: iteration-0 m/r and their x-shifts are host-precomputed and
DMA-loaded (edge-zero slivers ride along and persist, replacing the big
Pool memsets); per-xb inputs stream in urgency order so step (0,0) starts
~9us in. Steady state is DVE-bound (t_d TTs, 2x bf16 mode) at ~91% of the
kernel span with PE (accum+transposes) at ~88%.
"""

import sys

sys.path.insert(0, "/opt/trn_rl_repo")

import numpy as np

import concourse.bass as bass
import concourse.bacc as bacc
import concourse.tile as tile
from concourse import mybir
from concourse.bass_utils import run_bass_kernel_spmd

F32 = mybir.dt.float32
BF16 = mybir.dt.bfloat16

P = 128          # partitions
C = 21           # classes
RG = 6           # y-rows per C-packed group (6*21=126 partitions)
NB = 23          # row-blocks per slab (138 = 6*23)
YT = 138         # slab rows (128 own + 2*5 halo)
XB = 4           # x blocks (512 = 4*128)
D = 8            # directions
W = 512
HALO = 5
OWN = 128
MAX_ITER = 5
CP = RG * C      # 126
CCH = 7          # classes per PSUM accumulation chunk (3 chunks of 7*138)
DIRS = [(0, 1), (0, -1), (1, 0), (-1, 0), (1, 1), (1, -1), (-1, 1), (-1, -1)]

_CACHED_NC = None


def build_nc():
    nc = bacc.Bacc("TRN2")
    uuc_d = nc.dram_tensor("uuc", [P, XB, C, YT], BF16, kind="ExternalInput")
    mx0_d = nc.dram_tensor("mx0in", [P, XB, C, YT], BF16, kind="ExternalInput")
    mxp0_d = nc.dram_tensor("mxp0in", [P, XB, C, YT], BF16, kind="ExternalInput")
    mxm0_d = nc.dram_tensor("mxm0in", [P, XB, C, YT], BF16, kind="ExternalInput")
    r0_d = nc.dram_tensor("r0in", [P, XB, YT], BF16, kind="ExternalInput")
    rp0_d = nc.dram_tensor("rp0in", [P, XB, YT], BF16, kind="ExternalInput")
    rm0_d = nc.dram_tensor("rm0in", [P, XB, YT], BF16, kind="ExternalInput")
    ew_d = nc.dram_tensor("ew", [P, D, XB, YT], BF16, kind="ExternalInput")
    lcb_d = nc.dram_tensor("lcblk", [CP, CP], BF16, kind="ExternalInput")
    j6_d = nc.dram_tensor("j6", [CP, RG], BF16, kind="ExternalInput")
    ide_d = nc.dram_tensor("ident", [P, P], BF16, kind="ExternalInput")
    yout_d = nc.dram_tensor("yout", [P, XB, C, YT], BF16, kind="ExternalOutput")

    MUL = mybir.AluOpType.mult
    EXP = mybir.ActivationFunctionType.Exp

    with tile.TileContext(nc) as tc:
        with (
            tc.tile_pool(name="state", bufs=1) as st,
            tc.tile_pool(name="ecp", bufs=2) as ecp,
            tc.tile_pool(name="w2p", bufs=16) as wp,
            tc.tile_pool(name="t0a", bufs=3) as t0a,
            tc.tile_pool(name="t0b", bufs=3) as t0b,
            tc.tile_pool(name="tpa", bufs=4) as tpa,
            tc.tile_pool(name="tpb", bufs=4) as tpb,
            tc.tile_pool(name="tma", bufs=4) as tma,
            tc.tile_pool(name="tmb", bufs=3) as tmb,
            tc.tile_pool(name="uup", bufs=2) as up,
            tc.tile_pool(name="pt", bufs=1, space="PSUM") as pt,
            tc.tile_pool(name="pm", bufs=1, space="PSUM") as pm,
            tc.tile_pool(name="pss", bufs=2, space="PSUM") as pss,
            tc.tile_pool(name="pacc", bufs=2, space="PSUM") as pac,
        ):
            UU = st.tile([P, XB, C, YT], BF16)    # c-major
            EWs = st.tile([P, D, XB, YT], BF16)
            Y = st.tile([P, XB, YT, C], BF16)     # y-major
            MX0 = st.tile([P, XB, C, YT], BF16)
            MXP = st.tile([P, XB, C, YT], BF16)   # m shifted x+1
            MXM = st.tile([P, XB, C, YT], BF16)   # m shifted x-1
            S32 = st.tile([P, XB, YT], F32)
            R16 = st.tile([P, XB, YT], BF16)
            RP = st.tile([P, XB, YT], BF16)
            RM = st.tile([P, XB, YT], BF16)
            IDE = st.tile([P, P], BF16)
            LCB = st.tile([CP, CP], BF16)
            J6 = st.tile([CP, RG], BF16)

            # head-latency ordering: iteration-0 shift tensors come fully
            # host-precomputed (their DMA loads carry the image-edge zeros,
            # which persist: the it>=1 shift DMAs never write those edge
            # slivers). Step (0,0)'s inputs load first; bulk loads for the
            # later xbs are issued interleaved into the first steps so they
            # don't block step 0 on the serial DMA device.
            nc.sync.dma_start(out=EWs[:, :, 0, :], in_=ew_d[:, :, 0, :])
            nc.sync.dma_start(out=R16[:], in_=r0_d[:])
            nc.sync.dma_start(out=MX0[:, 0], in_=mx0_d[:, 0])
            nc.sync.dma_start(out=RP[:], in_=rp0_d[:])
            nc.sync.dma_start(out=RM[:], in_=rm0_d[:])
            nc.sync.dma_start(out=MXP[:, 0], in_=mxp0_d[:, 0])
            nc.sync.dma_start(out=IDE[:], in_=ide_d[:])
            nc.sync.dma_start(out=MXM[:, 0], in_=mxm0_d[:, 0])
            nc.sync.dma_start(out=UU[:, 0], in_=uuc_d[:, 0])
            nc.sync.dma_start(out=LCB[:], in_=lcb_d[:])
            nc.sync.dma_start(out=J6[:], in_=j6_d[:])

            # term-tile families per dy: the windowed muls never write the
            # edge row, so zero it once and the zero persists across pool
            # rotations (each family only ever hosts same-window terms).
            t_fam = {
                0: ((t0a, "t0a", 3), (t0b, "t0b", 3)),
                1: ((tpa, "tpa", 4), (tpb, "tpb", 4)),
                -1: ((tma, "tma", 4), (tmb, "tmb", 3)),
            }
            for dyk, ((pa, taga, na), (pb, tagb, nb)) in t_fam.items():
                if dyk == 0:
                    continue
                er = YT - 1 if dyk == 1 else 0
                for _ in range(na):
                    t = pa.tile([P, CCH, YT], BF16, tag=taga)
                    nc.gpsimd.memset(t[:, :, er : er + 1], 0)
                for _ in range(nb):
                    t = pb.tile([P, 2 * CCH, YT], BF16, tag=tagb)
                    nc.gpsimd.memset(t[:, :, er : er + 1], 0)

            def phase12(it, xb):
                """transpose Y -> C-packed, exp, LC/s matmuls, MX0 evac, r."""
                s_ps = pss.tile([P, YT], F32, tag="s")
                EC = ecp.tile([CP, NB, P], BF16, tag="ec")
                for rb0 in range(0, NB, 8):
                    nrb = min(8, NB - rb0)
                    yc = pt.tile([CP, 8 * P], BF16, tag="yc")
                    for k in range(nrb):
                        rb = rb0 + k
                        blk = Y[:, xb, rb * RG : (rb + 1) * RG, :]
                        nc.tensor.transpose(
                            out=yc[:, k * P : (k + 1) * P],
                            in_=blk.rearrange("p a b -> p (a b)"),
                            identity=IDE[:],
                        )
                    nc.scalar.activation(
                        out=EC[:, rb0 : rb0 + nrb, :],
                        in_=yc[:, 0 : nrb * P].rearrange("p (a b) -> p a b", b=P),
                        func=EXP, scale=-1.0,
                    )
                for rb0 in range(0, NB, 4):
                    nrb = min(4, NB - rb0)
                    mxp = pm.tile([P, 4 * CP], F32, tag="mxp")
                    for k in range(nrb):
                        rb = rb0 + k
                        ecs = EC[:, rb, :]
                        nc.tensor.matmul(
                            out=mxp[:, k * CP : (k + 1) * CP],
                            lhsT=ecs, rhs=LCB[:], start=True, stop=True,
                        )
                        nc.tensor.matmul(
                            out=s_ps[:, rb * RG : (rb + 1) * RG],
                            lhsT=ecs, rhs=J6[:], start=True, stop=True,
                        )
                    nc.scalar.copy(
                        out=MX0[:, xb, :, rb0 * RG : (rb0 + nrb) * RG]
                        .rearrange("p c (a b) -> p a c b", b=RG),
                        in_=mxp[:, 0 : nrb * CP].rearrange(
                            "p (a c b) -> p a c b", c=C, b=RG
                        ),
                    )
                nc.vector.reciprocal(out=S32[:, xb], in_=s_ps[:])
                nc.scalar.copy(out=R16[:, xb], in_=S32[:, xb])

            def mulaccum(it, xb):
                """per-xb shifts, weights, weighted terms, PSUM accumulation."""
                last = it == MAX_ITER - 1
                # x+-1 shifted r and m slices (edges from neighbor xb;
                # image-edge columns keep the zeros loaded with the it=0
                # host-precomputed shift tensors). it=0 is fully host-fed.
                if it > 0:
                    nc.sync.dma_start(out=RP[0 : P - 1, xb], in_=R16[1:P, xb])
                    nc.sync.dma_start(out=RM[1:P, xb], in_=R16[0 : P - 1, xb])
                    # shifts split into A/B class halves: the A TTs only
                    # need classes 0:7, so they start ~1.3us earlier than
                    # behind a whole-tensor shift
                    nc.sync.dma_start(
                        out=MXP[0 : P - 1, xb, 0:CCH], in_=MX0[1:P, xb, 0:CCH]
                    )
                    nc.sync.dma_start(
                        out=MXM[1:P, xb, 0:CCH], in_=MX0[0 : P - 1, xb, 0:CCH]
                    )
                    nc.sync.dma_start(
                        out=MXP[0 : P - 1, xb, CCH:C], in_=MX0[1:P, xb, CCH:C]
                    )
                    nc.sync.dma_start(
                        out=MXM[1:P, xb, CCH:C], in_=MX0[0 : P - 1, xb, CCH:C]
                    )
                    if xb < XB - 1:
                        nc.sync.dma_start(
                            out=RP[P - 1 : P, xb], in_=R16[0:1, xb + 1]
                        )
                        nc.sync.dma_start(
                            out=MXP[P - 1 : P, xb], in_=MX0[0:1, xb + 1]
                        )
                    if xb > 0:
                        nc.sync.dma_start(
                            out=RM[0:1, xb], in_=R16[P - 1 : P, xb - 1]
                        )
                        nc.sync.dma_start(
                            out=MXM[0:1, xb], in_=MX0[P - 1 : P, xb - 1]
                        )
                w2s = {}
                for d in (2, 3, 0, 4, 6, 1, 5, 7):
                    dy, dx = DIRS[d]
                    rsrc = {1: RP, 0: R16, -1: RM}[dx]
                    a, b = max(0, -dy), min(YT, YT - dy)
                    w2 = wp.tile([P, YT], BF16, tag="w2")
                    nc.gpsimd.tensor_tensor(
                        out=w2[:, a:b],
                        in0=EWs[:, d, xb, a:b],
                        in1=rsrc[:, xb, a + dy : b + dy],
                        op=MUL,
                    )
                    w2s[d] = w2
                def ttA(d):
                    dy, dx = DIRS[d]
                    mx = {1: MXP, 0: MX0, -1: MXM}[dx]
                    a, b = max(0, -dy), min(YT, YT - dy)
                    (pa, taga, _na), _pb = t_fam[dy]
                    tA = pa.tile([P, CCH, YT], BF16, tag=taga)
                    nc.vector.tensor_tensor(
                        out=tA[:, :, a:b],
                        in0=w2s[d][:, a:b]
                        .unsqueeze(1)
                        .broadcast_to((P, CCH, b - a)),
                        in1=mx[:, xb, 0:CCH, a + dy : b + dy],
                        op=MUL,
                    )
                    return tA

                def ttB(d):
                    dy, dx = DIRS[d]
                    mx = {1: MXP, 0: MX0, -1: MXM}[dx]
                    a, b = max(0, -dy), min(YT, YT - dy)
                    _pa, (pb, tagb, _nb) = t_fam[dy]
                    tB = pb.tile([P, 2 * CCH, YT], BF16, tag=tagb)
                    nc.vector.tensor_tensor(
                        out=tB[:, :, a:b],
                        in0=w2s[d][:, a:b]
                        .unsqueeze(1)
                        .broadcast_to((P, 2 * CCH, b - a)),
                        in1=mx[:, xb, CCH:C, a + dy : b + dy],
                        op=MUL,
                    )
                    return tB

                if last and xb >= XB - 2:
                    # very last step: fine-grained per-chunk terms so the
                    # final PE accumulation interleaves with the DVE TTs
                    # instead of trailing them by a full chunk
                    for c0 in range(0, C, CCH):
                        terms = []
                        for d in (2, 3, 0, 4, 6, 1, 5, 7):
                            dy, dx = DIRS[d]
                            mx = {1: MXP, 0: MX0, -1: MXM}[dx]
                            a, b = max(0, -dy), min(YT, YT - dy)
                            (pa, taga, _na), _pb = t_fam[dy]
                            t = pa.tile([P, CCH, YT], BF16, tag=taga)
                            nc.vector.tensor_tensor(
                                out=t[:, :, a:b],
                                in0=w2s[d][:, a:b]
                                .unsqueeze(1)
                                .broadcast_to((P, CCH, b - a)),
                                in1=mx[:, xb, c0 : c0 + CCH, a + dy : b + dy],
                                op=MUL,
                            )
                            terms.append(t)
                        rhss = [UU[:, xb, c0 : c0 + CCH, :]] + [
                            t[:] for t in terms
                        ]
                        acc = pac.tile([P, CCH * YT], F32, tag="acc")
                        NF = CCH * YT
                        for ti, rt in enumerate(rhss):
                            rfl = rt.rearrange("p a b -> p (a b)")
                            for f0 in range(0, NF, 512):
                                f1 = min(f0 + 512, NF)
                                nc.tensor.matmul(
                                    out=acc[:, f0:f1],
                                    lhsT=IDE[:],
                                    rhs=rfl[:, f0:f1],
                                    start=(ti == 0),
                                    stop=(ti == len(rhss) - 1),
                                )
                        yo = up.tile([P, CCH, YT], BF16, tag="yo")
                        nc.scalar.copy(
                            out=yo[:],
                            in_=acc[:].rearrange("p (a b) -> p a b", b=YT),
                        )
                        nc.sync.dma_start(
                            out=yout_d[:, xb, c0 : c0 + CCH, :], in_=yo[:]
                        )
                    return
                tAs, tBs = {}, {}
                if last:
                    for d in (2, 3, 0, 4, 6, 1, 5, 7):
                        tBs[d] = ttB(d)
                        tAs[d] = ttA(d)
                else:
                    # all-A-first: chunk 0's accumulation depends only on
                    # the A tiles, so it starts ~4.5us into the step and
                    # overlaps the B TTs (interleaved A/B would stall it
                    # until the eighth A at ~12us)
                    for d in (2, 3, 0, 4, 6, 1, 5, 7):
                        tAs[d] = ttA(d)
                    for d in (2, 3, 0, 4, 6, 1, 5, 7):
                        tBs[d] = ttB(d)
                for c0 in (7, 14, 0) if last else (0, 7, 14):
                    if c0 == 0:
                        terms = [tAs[d][:] for d in (2, 3, 0, 4, 6, 1, 5, 7)]
                    else:
                        s0 = c0 - CCH
                        terms = [
                            tBs[d][:, s0 : s0 + CCH, :]
                            for d in (2, 3, 0, 4, 6, 1, 5, 7)
                        ]
                    rhss = [UU[:, xb, c0 : c0 + CCH, :]] + terms
                    acc = pac.tile([P, CCH * YT], F32, tag="acc")
                    NF = CCH * YT
                    for ti, rt in enumerate(rhss):
                        rfl = rt.rearrange("p a b -> p (a b)")
                        for f0 in range(0, NF, 512):
                            f1 = min(f0 + 512, NF)
                            nc.tensor.matmul(
                                out=acc[:, f0:f1],
                                lhsT=IDE[:],
                                rhs=rfl[:, f0:f1],
                                start=(ti == 0),
                                stop=(ti == len(rhss) - 1),
                            )
                    if last:
                        yo = up.tile([P, CCH, YT], BF16, tag="yo")
                        nc.scalar.copy(
                            out=yo[:],
                            in_=acc[:].rearrange("p (a b) -> p a b", b=YT),
                        )
                        nc.sync.dma_start(
                            out=yout_d[:, xb, c0 : c0 + CCH, :], in_=yo[:]
                        )
                    else:
                        nc.scalar.copy(
                            out=Y[:, xb, :, c0 : c0 + CCH].rearrange(
                                "p y c -> p c y"
                            ),
                            in_=acc[:].rearrange("p (c y) -> p c y", y=YT),
                        )

            # software pipeline: phase12 of step s issues three steps
            # ahead (its only input, Y[:, xb], is written four steps
            # earlier), giving the long m-chain (transpose->exp->LC->
            # evac->shift->w2) a full extra step of slack. mulaccum must
            # precede phase12 within an iteration: the same-iteration
            # phase12 overwrites MX0/R16 slices whose edge columns the
            # mulaccum still reads.
            ma_steps = [(it, xb) for it in range(MAX_ITER) for xb in range(XB)]
            ph_steps = [(it, xb) for it in range(1, MAX_ITER) for xb in range(XB)]
            # deferred bulk loads: each later xb's it=0 inputs are issued
            # one step ahead of the step that consumes them, so they don't
            # delay step 0 on the serial DMA device.
            def xb_set(xb_):
                # ew first: it gates the Pool w2s, which gate every TT of
                # the step (m tensors are only needed once the TTs start)
                return [
                    (EWs, None, xb_),
                    (MX0, mx0_d, xb_),
                    (MXP, mxp0_d, xb_),
                    (MXM, mxm0_d, xb_),
                ]

            deferred = {
                0: xb_set(1) + [(UU, uuc_d, 1)],
                1: xb_set(2) + [(UU, uuc_d, 2)],
                2: xb_set(3) + [(UU, uuc_d, 3)],
            }
            for k in range(len(ma_steps)):
                for dst, srcd, xb_ in deferred.get(k, ()):
                    if srcd is None:
                        nc.sync.dma_start(
                            out=EWs[:, :, xb_, :], in_=ew_d[:, :, xb_, :]
                        )
                    else:
                        nc.sync.dma_start(out=dst[:, xb_], in_=srcd[:, xb_])
                mulaccum(*ma_steps[k])
                if 1 <= k and k - 1 < len(ph_steps):
                    phase12(*ph_steps[k - 1])

    nc.finalize()
    return nc


def _prep_core(u, ew, b, hc):
    y0 = 128 * hc
    ys = min(max(y0 - HALO, 0), 512 - YT)
    u_slab = u[b, 0, :, ys : ys + YT, :]          # [21, 138, 512]
    ew_slab = ew[b, :, ys : ys + YT, :]           # [8, 138, 512]
    uuc = np.ascontiguousarray(
        u_slab.reshape(C, YT, XB, P).transpose(3, 2, 0, 1), dtype=np.float32
    )                                             # [P, XB, C, YT]
    ewp = np.ascontiguousarray(
        ew_slab.reshape(D, YT, XB, P).transpose(3, 0, 2, 1)
    )                                             # [P, D, XB, YT]
    return uuc, ewp, ys, y0 - ys


def kernel(unary, edge_weights, label_context, _trace=False, _tmpdir=None):
    global _CACHED_NC
    if _CACHED_NC is None:
        _CACHED_NC = build_nc()
    nc = _CACHED_NC

    import ml_dtypes

    bf16 = ml_dtypes.bfloat16

    u = np.asarray(unary, dtype=np.float32)
    ew = np.asarray(edge_weights, dtype=np.float32)
    lc = np.asarray(label_context, dtype=np.float32)

    # C-packed row index is (j, k) = y-within-group-major, class-minor:
    # p_in = j*21 + k.  LCB columns are (l, j2): p_out = l*6 + j2.
    # LCB[(j,k),(l,j2)] = LC[l,k]/8 * I6[j,j2]
    lcb = np.einsum(
        "jm,lk->jklm", np.eye(RG, dtype=np.float32), lc / 8.0
    ).reshape(CP, CP).astype(bf16)
    j6 = np.einsum(
        "jm,k->jkm", np.eye(RG, dtype=np.float32), np.ones(C, np.float32)
    ).reshape(CP, RG).astype(bf16)
    ident = np.eye(P, dtype=np.float32).astype(bf16)

    in_maps = []
    offs = []
    for core in range(8):
        b, hc = core // 4, core % 4
        uuc, ewp, ys, off = _prep_core(u, ew, b, hc)
        offs.append(off)
        uuc16 = uuc.astype(bf16)
        # iteration-0 phase12 on the host: E = exp(-u), m = (LC/8) @ E,
        # r = 1 / sum_c E, rounded at the same points as the device path
        e16f = np.exp(-uuc16.astype(np.float32)).astype(bf16).astype(np.float32)
        lcf = (lc / 8.0).astype(bf16).astype(np.float32)
        m0 = np.einsum("lk,pxky->pxly", lcf, e16f).astype(bf16)
        r0 = (1.0 / e16f.sum(axis=2)).astype(bf16)

        # host-precomputed x+-1 shifts of m0/r0 (global x = xb*128 + p;
        # zero beyond the image edge), matching the device DMA shifts
        def shx(t, s):  # t: [P, XB, ...], shift source x+s
            f = np.moveaxis(t, 1, 0).reshape((XB * P,) + t.shape[2:])
            o = np.zeros_like(f)
            if s == 1:
                o[: XB * P - 1] = f[1:]
            else:
                o[1:] = f[: XB * P - 1]
            return np.moveaxis(o.reshape((XB, P) + t.shape[2:]), 0, 1)

        mxp0 = shx(m0, 1)
        mxm0 = shx(m0, -1)
        rp0 = shx(r0, 1)
        rm0 = shx(r0, -1)
        in_maps.append(
            {
                "uuc": uuc16,
                "mx0in": m0,
                "mxp0in": mxp0,
                "mxm0in": mxm0,
                "r0in": r0,
                "rp0in": rp0,
                "rm0in": rm0,
                "ew": ewp.astype(bf16),
                "lcblk": lcb,
                "j6": j6,
                "ident": ident,
            }
        )

    kwargs = {}
    if _trace:
        kwargs = dict(trace=True, trace_cores=[0], tmpdir=_tmpdir)
    res = run_bass_kernel_spmd(nc, in_maps, core_ids=list(range(8)), **kwargs)

    out = np.zeros((2, 1, C, 512, 512), dtype=np.float32)
    for core in range(8):
        b, hc = core // 4, core % 4
        yo = res.results[core]["yout"]            # [P, XB, C, YT] bf16
        slab = yo.astype(np.float32).transpose(2, 3, 1, 0).reshape(C, YT, W)
        off = offs[core]
        out[b, 0, :, 128 * hc : 128 * (hc + 1), :] = slab[:, off : off + OWN, :]
    if _trace:
        return out, res
    return out



# revision 63
# speedup vs baseline: 1.0220x; 1.0220x over previous
"""MeanField CRF message-passing kernel for 8 Trainium2 NeuronCores.

Sharding: (B=2) x (H into 4 chunks of 128 rows) = 8 slabs, each with a
5-row halo on slab-interior edges (5 mean-field iterations x 1-row
stencil reach), so cores run fully independently (no collectives).

Per-core layouts (bf16 state for DVE 2x / PE 1-cycle-per-row modes):
  y-major : [x mod 128 -> partitions, (xblock, y, class) -> free]  (Y only)
  c-major : [x mod 128 -> partitions, (xblock, class, y) -> free]
  C-packed: [(y mod 6, class) -> 126 partitions, x -> free]
Math per iteration (equivalent-transformed from the reference):
  YC   = PE-transpose(Y)                 (bf16 PSUM; Y y-major so the
                                          transpose input is a flat AP)
  EC   = exp(-YC)                        (ACT reads PSUM, fuses evac)
  m    = ECslice^T @ LCB2 ; s = ECslice^T @ J6   (PE; fuses LC^T/8
                                          contraction with transpose back)
  r    = 1/s                             (DVE), folded into w2
  mxp/mxm = x+-1 partition-shifted m     (DMA, idle engine)
  w2_d = ew_d * shift_d(r)               (DVE TT, bf16 2x)
  t_d  = w2_d(bcast C) * shift_d(m)      (DVE TT, bf16 2x, c-major)
  Ypsum= I^T@u + sum_d I^T@t_d           (PE identity-matmul PSUM accum
                                          -> replaces all DVE adds)
  Y    = evac(Ypsum)                     (ACT, scatter to y-major bf16)
Final cost = Y after iteration 5, stored bf16 (~2.4e-3 rel err, gate 2e-2).

Head-latency: iteration-0 m/r and their x-shifts are host-precomputed and
DMA-loaded (edge-zero slivers ride along and persist, replacing the big
Pool memsets); per-xb inputs stream in urgency order so step (0,0) starts
~9us in. Steady state is DVE-bound (t_d TTs, 2x bf16 mode) at ~91% of the
kernel span with PE (accum+transposes) at ~88%.
"""

import sys

sys.path.insert(0, "/opt/trn_rl_repo")

import numpy as np

import concourse.bass as bass
import concourse.bacc as bacc
import concourse.tile as tile
from concourse import mybir
from concourse.bass_utils import run_bass_kernel_spmd

F32 = mybir.dt.float32
BF16 = mybir.dt.bfloat16

P = 128          # partitions
C = 21           # classes
RG = 6           # y-rows per C-packed group (6*21=126 partitions)
NB = 23          # row-blocks per slab (138 = 6*23)
YT = 138         # slab rows (128 own + 2*5 halo)
XB = 4           # x blocks (512 = 4*128)
D = 8            # directions
W = 512
HALO = 5
OWN = 128
MAX_ITER = 5
CP = RG * C      # 126
CCH = 7          # classes per PSUM accumulation chunk (3 chunks of 7*138)
DIRS = [(0, 1), (0, -1), (1, 0), (-1, 0), (1, 1), (1, -1), (-1, 1), (-1, -1)]

_CACHED_NC = None


def build_nc():
    nc = bacc.Bacc("TRN2")
    uuc_d = nc.dram_tensor("uuc", [P, XB, C, YT], BF16, kind="ExternalInput")
    mx0_d = nc.dram_tensor("mx0in", [P, XB, C, YT], BF16, kind="ExternalInput")
    mxp0_d = nc.dram_tensor("mxp0in", [P, XB, C, YT], BF16, kind="ExternalInput")
    mxm0_d = nc.dram_tensor("mxm0in", [P, XB, C, YT], BF16, kind="ExternalInput")
    r0_d = nc.dram_tensor("r0in", [P, XB, YT], BF16, kind="ExternalInput")
    rp0_d = nc.dram_tensor("rp0in", [P, XB, YT], BF16, kind="ExternalInput")
    rm0_d = nc.dram_tensor("rm0in", [P, XB, YT], BF16, kind="ExternalInput")
    ew_d = nc.dram_tensor("ew", [P, D, XB, YT], BF16, kind="ExternalInput")
    lcb_d = nc.dram_tensor("lcblk", [CP, CP], BF16, kind="ExternalInput")
    j6_d = nc.dram_tensor("j6", [CP, RG], BF16, kind="ExternalInput")
    ide_d = nc.dram_tensor("ident", [P, P], BF16, kind="ExternalInput")
    yout_d = nc.dram_tensor("yout", [P, XB, C, YT], BF16, kind="ExternalOutput")

    MUL = mybir.AluOpType.mult
    EXP = mybir.ActivationFunctionType.Exp

    with tile.TileContext(nc) as tc:
        with (
            tc.tile_pool(name="state", bufs=1) as st,
            tc.tile_pool(name="ecp", bufs=2) as ecp,
            tc.tile_pool(name="w2p", bufs=16) as wp,
            tc.tile_pool(name="t0a", bufs=3) as t0a,
            tc.tile_pool(name="t0b", bufs=3) as t0b,
            tc.tile_pool(name="tpa", bufs=4) as tpa,
            tc.tile_pool(name="tpb", bufs=4) as tpb,
            tc.tile_pool(name="tma", bufs=4) as tma,
            tc.tile_pool(name="tmb", bufs=3) as tmb,
            tc.tile_pool(name="uup", bufs=2) as up,
            tc.tile_pool(name="pt", bufs=1, space="PSUM") as pt,
            tc.tile_pool(name="pm", bufs=2, space="PSUM") as pm,
            tc.tile_pool(name="pss", bufs=1, space="PSUM") as pss,
            tc.tile_pool(name="pacc", bufs=2, space="PSUM") as pac,
        ):
            UU = st.tile([P, XB, C, YT], BF16)    # c-major
            EWs = st.tile([P, D, XB, YT], BF16)
            Y = st.tile([P, XB, YT, C], BF16)     # y-major
            MX0 = st.tile([P, XB, C, YT], BF16)
            MXP = st.tile([P, XB, C, YT], BF16)   # m shifted x+1
            MXM = st.tile([P, XB, C, YT], BF16)   # m shifted x-1
            S32 = st.tile([P, XB, YT], F32)
            R16 = st.tile([P, XB, YT], BF16)
            RP = st.tile([P, XB, YT], BF16)
            RM = st.tile([P, XB, YT], BF16)
            IDE = st.tile([P, P], BF16)
            LCB = st.tile([CP, CP], BF16)
            J6 = st.tile([CP, RG], BF16)

            # head-latency ordering: iteration-0 shift tensors come fully
            # host-precomputed (their DMA loads carry the image-edge zeros,
            # which persist: the it>=1 shift DMAs never write those edge
            # slivers). Step (0,0)'s inputs load first; bulk loads for the
            # later xbs are issued interleaved into the first steps so they
            # don't block step 0 on the serial DMA device.
            nc.sync.dma_start(out=EWs[:, :, 0, :], in_=ew_d[:, :, 0, :])
            nc.sync.dma_start(out=R16[:], in_=r0_d[:])
            nc.sync.dma_start(out=MX0[:, 0], in_=mx0_d[:, 0])
            nc.sync.dma_start(out=RP[:], in_=rp0_d[:])
            nc.sync.dma_start(out=RM[:], in_=rm0_d[:])
            nc.sync.dma_start(out=MXP[:, 0], in_=mxp0_d[:, 0])
            nc.sync.dma_start(out=IDE[:], in_=ide_d[:])
            nc.sync.dma_start(out=MXM[:, 0], in_=mxm0_d[:, 0])
            nc.sync.dma_start(out=UU[:, 0], in_=uuc_d[:, 0])
            nc.sync.dma_start(out=LCB[:], in_=lcb_d[:])
            nc.sync.dma_start(out=J6[:], in_=j6_d[:])

            # term-tile families per dy: the windowed muls never write the
            # edge row, so zero it once and the zero persists across pool
            # rotations (each family only ever hosts same-window terms).
            t_fam = {
                0: ((t0a, "t0a", 3), (t0b, "t0b", 3)),
                1: ((tpa, "tpa", 4), (tpb, "tpb", 4)),
                -1: ((tma, "tma", 4), (tmb, "tmb", 3)),
            }
            for dyk, ((pa, taga, na), (pb, tagb, nb)) in t_fam.items():
                if dyk == 0:
                    continue
                er = YT - 1 if dyk == 1 else 0
                for _ in range(na):
                    t = pa.tile([P, CCH, YT], BF16, tag=taga)
                    nc.gpsimd.memset(t[:, :, er : er + 1], 0)
                for _ in range(nb):
                    t = pb.tile([P, 2 * CCH, YT], BF16, tag=tagb)
                    nc.gpsimd.memset(t[:, :, er : er + 1], 0)

            def phase12(it, xb):
                """transpose Y -> C-packed, exp, LC/s matmuls, MX0 evac, r."""
                s_ps = pss.tile([P, YT], F32, tag="s")
                EC = ecp.tile([CP, NB, P], BF16, tag="ec")
                for rb0 in range(0, NB, 8):
                    nrb = min(8, NB - rb0)
                    yc = pt.tile([CP, 8 * P], BF16, tag="yc")
                    for k in range(nrb):
                        rb = rb0 + k
                        blk = Y[:, xb, rb * RG : (rb + 1) * RG, :]
                        nc.tensor.transpose(
                            out=yc[:, k * P : (k + 1) * P],
                            in_=blk.rearrange("p a b -> p (a b)"),
                            identity=IDE[:],
                        )
                    nc.scalar.activation(
                        out=EC[:, rb0 : rb0 + nrb, :],
                        in_=yc[:, 0 : nrb * P].rearrange("p (a b) -> p a b", b=P),
                        func=EXP, scale=-1.0,
                    )
                for rb0 in range(0, NB, 4):
                    nrb = min(4, NB - rb0)
                    mxp = pm.tile([P, 4 * CP], F32, tag="mxp")
                    for k in range(nrb):
                        rb = rb0 + k
                        ecs = EC[:, rb, :]
                        nc.tensor.matmul(
                            out=mxp[:, k * CP : (k + 1) * CP],
                            lhsT=ecs, rhs=LCB[:], start=True, stop=True,
                        )
                        nc.tensor.matmul(
                            out=s_ps[:, rb * RG : (rb + 1) * RG],
                            lhsT=ecs, rhs=J6[:], start=True, stop=True,
                        )
                    nc.scalar.copy(
                        out=MX0[:, xb, :, rb0 * RG : (rb0 + nrb) * RG]
                        .rearrange("p c (a b) -> p a c b", b=RG),
                        in_=mxp[:, 0 : nrb * CP].rearrange(
                            "p (a c b) -> p a c b", c=C, b=RG
                        ),
                    )
                nc.vector.reciprocal(out=S32[:, xb], in_=s_ps[:])
                nc.scalar.copy(out=R16[:, xb], in_=S32[:, xb])

            def mulaccum(it, xb):
                """per-xb shifts, weights, weighted terms, PSUM accumulation."""
                last = it == MAX_ITER - 1
                # x+-1 shifted r and m slices (edges from neighbor xb;
                # image-edge columns keep the zeros loaded with the it=0
                # host-precomputed shift tensors). it=0 is fully host-fed.
                if it > 0:
                    nc.sync.dma_start(out=RP[0 : P - 1, xb], in_=R16[1:P, xb])
                    nc.sync.dma_start(out=RM[1:P, xb], in_=R16[0 : P - 1, xb])
                    # shifts split into A/B class halves: the A TTs only
                    # need classes 0:7, so they start ~1.3us earlier than
                    # behind a whole-tensor shift
                    nc.sync.dma_start(
                        out=MXP[0 : P - 1, xb, 0:CCH], in_=MX0[1:P, xb, 0:CCH]
                    )
                    nc.sync.dma_start(
                        out=MXM[1:P, xb, 0:CCH], in_=MX0[0 : P - 1, xb, 0:CCH]
                    )
                    nc.sync.dma_start(
                        out=MXP[0 : P - 1, xb, CCH:C], in_=MX0[1:P, xb, CCH:C]
                    )
                    nc.sync.dma_start(
                        out=MXM[1:P, xb, CCH:C], in_=MX0[0 : P - 1, xb, CCH:C]
                    )
                    if xb < XB - 1:
                        nc.sync.dma_start(
                            out=RP[P - 1 : P, xb], in_=R16[0:1, xb + 1]
                        )
                        nc.sync.dma_start(
                            out=MXP[P - 1 : P, xb], in_=MX0[0:1, xb + 1]
                        )
                    if xb > 0:
                        nc.sync.dma_start(
                            out=RM[0:1, xb], in_=R16[P - 1 : P, xb - 1]
                        )
                        nc.sync.dma_start(
                            out=MXM[0:1, xb], in_=MX0[P - 1 : P, xb - 1]
                        )
                w2s = {}
                for d in (2, 3, 0, 4, 6, 1, 5, 7):
                    dy, dx = DIRS[d]
                    rsrc = {1: RP, 0: R16, -1: RM}[dx]
                    a, b = max(0, -dy), min(YT, YT - dy)
                    w2 = wp.tile([P, YT], BF16, tag="w2")
                    nc.gpsimd.tensor_tensor(
                        out=w2[:, a:b],
                        in0=EWs[:, d, xb, a:b],
                        in1=rsrc[:, xb, a + dy : b + dy],
                        op=MUL,
                    )
                    w2s[d] = w2
                def ttA(d):
                    dy, dx = DIRS[d]
                    mx = {1: MXP, 0: MX0, -1: MXM}[dx]
                    a, b = max(0, -dy), min(YT, YT - dy)
                    (pa, taga, _na), _pb = t_fam[dy]
                    tA = pa.tile([P, CCH, YT], BF16, tag=taga)
                    nc.vector.tensor_tensor(
                        out=tA[:, :, a:b],
                        in0=w2s[d][:, a:b]
                        .unsqueeze(1)
                        .broadcast_to((P, CCH, b - a)),
                        in1=mx[:, xb, 0:CCH, a + dy : b + dy],
                        op=MUL,
                    )
                    return tA

                def ttB(d):
                    dy, dx = DIRS[d]
                    mx = {1: MXP, 0: MX0, -1: MXM}[dx]
                    a, b = max(0, -dy), min(YT, YT - dy)
                    _pa, (pb, tagb, _nb) = t_fam[dy]
                    tB = pb.tile([P, 2 * CCH, YT], BF16, tag=tagb)
                    nc.vector.tensor_tensor(
                        out=tB[:, :, a:b],
                        in0=w2s[d][:, a:b]
                        .unsqueeze(1)
                        .broadcast_to((P, 2 * CCH, b - a)),
                        in1=mx[:, xb, CCH:C, a + dy : b + dy],
                        op=MUL,
                    )
                    return tB

                if last and xb >= XB - 2:
                    # very last step: fine-grained per-chunk terms so the
                    # final PE accumulation interleaves with the DVE TTs
                    # instead of trailing them by a full chunk
                    for c0 in range(0, C, CCH):
                        terms = []
                        for d in (2, 3, 0, 4, 6, 1, 5, 7):
                            dy, dx = DIRS[d]
                            mx = {1: MXP, 0: MX0, -1: MXM}[dx]
                            a, b = max(0, -dy), min(YT, YT - dy)
                            (pa, taga, _na), _pb = t_fam[dy]
                            t = pa.tile([P, CCH, YT], BF16, tag=taga)
                            nc.vector.tensor_tensor(
                                out=t[:, :, a:b],
                                in0=w2s[d][:, a:b]
                                .unsqueeze(1)
                                .broadcast_to((P, CCH, b - a)),
                                in1=mx[:, xb, c0 : c0 + CCH, a + dy : b + dy],
                                op=MUL,
                            )
                            terms.append(t)
                        rhss = [UU[:, xb, c0 : c0 + CCH, :]] + [
                            t[:] for t in terms
                        ]
                        acc = pac.tile([P, CCH * YT], F32, tag="acc")
                        NF = CCH * YT
                        for ti, rt in enumerate(rhss):
                            rfl = rt.rearrange("p a b -> p (a b)")
                            for f0 in range(0, NF, 512):
                                f1 = min(f0 + 512, NF)
                                nc.tensor.matmul(
                                    out=acc[:, f0:f1],
                                    lhsT=IDE[:],
                                    rhs=rfl[:, f0:f1],
                                    start=(ti == 0),
                                    stop=(ti == len(rhss) - 1),
                                )
                        yo = up.tile([P, CCH, YT], BF16, tag="yo")
                        nc.scalar.copy(
                            out=yo[:],
                            in_=acc[:].rearrange("p (a b) -> p a b", b=YT),
                        )
                        nc.sync.dma_start(
                            out=yout_d[:, xb, c0 : c0 + CCH, :], in_=yo[:]
                        )
                    return
                tAs, tBs = {}, {}
                if last:
                    for d in (2, 3, 0, 4, 6, 1, 5, 7):
                        tBs[d] = ttB(d)
                        tAs[d] = ttA(d)
                else:
                    # all-A-first: chunk 0's accumulation depends only on
                    # the A tiles, so it starts ~4.5us into the step and
                    # overlaps the B TTs (interleaved A/B would stall it
                    # until the eighth A at ~12us)
                    for d in (2, 3, 0, 4, 6, 1, 5, 7):
                        tAs[d] = ttA(d)
                    for d in (2, 3, 0, 4, 6, 1, 5, 7):
                        tBs[d] = ttB(d)
                for c0 in (7, 14, 0) if last else (0, 7, 14):
                    if c0 == 0:
                        terms = [tAs[d][:] for d in (2, 3, 0, 4, 6, 1, 5, 7)]
                    else:
                        s0 = c0 - CCH
                        terms = [
                            tBs[d][:, s0 : s0 + CCH, :]
                            for d in (2, 3, 0, 4, 6, 1, 5, 7)
                        ]
                    rhss = [UU[:, xb, c0 : c0 + CCH, :]] + terms
                    acc = pac.tile([P, CCH * YT], F32, tag="acc")
                    NF = CCH * YT
                    for ti, rt in enumerate(rhss):
                        rfl = rt.rearrange("p a b -> p (a b)")
                        for f0 in range(0, NF, 512):
                            f1 = min(f0 + 512, NF)
                            nc.tensor.matmul(
                                out=acc[:, f0:f1],
                                lhsT=IDE[:],
                                rhs=rfl[:, f0:f1],
                                start=(ti == 0),
                                stop=(ti == len(rhss) - 1),
                            )
                    if last:
                        yo = up.tile([P, CCH, YT], BF16, tag="yo")
                        nc.scalar.copy(
                            out=yo[:],
                            in_=acc[:].rearrange("p (a b) -> p a b", b=YT),
                        )
                        nc.sync.dma_start(
                            out=yout_d[:, xb, c0 : c0 + CCH, :], in_=yo[:]
                        )
                    else:
                        nc.scalar.copy(
                            out=Y[:, xb, :, c0 : c0 + CCH].rearrange(
                                "p y c -> p c y"
                            ),
                            in_=acc[:].rearrange("p (c y) -> p c y", y=YT),
                        )

            # software pipeline: phase12 of step s issues three steps
            # ahead (its only input, Y[:, xb], is written four steps
            # earlier), giving the long m-chain (transpose->exp->LC->
            # evac->shift->w2) a full extra step of slack. mulaccum must
            # precede phase12 within an iteration: the same-iteration
            # phase12 overwrites MX0/R16 slices whose edge columns the
            # mulaccum still reads.
            ma_steps = [(it, xb) for it in range(MAX_ITER) for xb in range(XB)]
            ph_steps = [(it, xb) for it in range(1, MAX_ITER) for xb in range(XB)]
            # deferred bulk loads: each later xb's it=0 inputs are issued
            # one step ahead of the step that consumes them, so they don't
            # delay step 0 on the serial DMA device.
            def xb_set(xb_):
                # ew first: it gates the Pool w2s, which gate every TT of
                # the step (m tensors are only needed once the TTs start)
                return [
                    (EWs, None, xb_),
                    (MX0, mx0_d, xb_),
                    (MXP, mxp0_d, xb_),
                    (MXM, mxm0_d, xb_),
                ]

            deferred = {
                0: xb_set(1) + [(UU, uuc_d, 1)],
                1: xb_set(2) + [(UU, uuc_d, 2)],
                2: xb_set(3) + [(UU, uuc_d, 3)],
            }
            for k in range(len(ma_steps)):
                for dst, srcd, xb_ in deferred.get(k, ()):
                    if srcd is None:
                        nc.sync.dma_start(
                            out=EWs[:, :, xb_, :], in_=ew_d[:, :, xb_, :]
                        )
                    else:
                        nc.sync.dma_start(out=dst[:, xb_], in_=srcd[:, xb_])
                mulaccum(*ma_steps[k])
                if 1 <= k and k - 1 < len(ph_steps):
                    phase12(*ph_steps[k - 1])

    nc.finalize()
    return nc


def _prep_core(u, ew, b, hc):
    y0 = 128 * hc
    ys = min(max(y0 - HALO, 0), 512 - YT)
    u_slab = u[b, 0, :, ys : ys + YT, :]          # [21, 138, 512]
    ew_slab = ew[b, :, ys : ys + YT, :]           # [8, 138, 512]
    uuc = np.ascontiguousarray(
        u_slab.reshape(C, YT, XB, P).transpose(3, 2, 0, 1), dtype=np.float32
    )                                             # [P, XB, C, YT]
    ewp = np.ascontiguousarray(
        ew_slab.reshape(D, YT, XB, P).transpose(3, 0, 2, 1)
    )                                             # [P, D, XB, YT]
    return uuc, ewp, ys, y0 - ys


def kernel(unary, edge_weights, label_context, _trace=False, _tmpdir=None):
    global _CACHED_NC
    if _CACHED_NC is None:
        _CACHED_NC = build_nc()
    nc = _CACHED_NC

    import ml_dtypes

    bf16 = ml_dtypes.bfloat16

    u = np.asarray(unary, dtype=np.float32)
    ew = np.asarray(edge_weights, dtype=np.float32)
    lc = np.asarray(label_context, dtype=np.float32)

    # C-packed row index is (j, k) = y-within-group-major, class-minor:
    # p_in = j*21 + k.  LCB columns are (l, j2): p_out = l*6 + j2.
    # LCB[(j,k),(l,j2)] = LC[l,k]/8 * I6[j,j2]
    lcb = np.einsum(
        "jm,lk->jklm", np.eye(RG, dtype=np.float32), lc / 8.0
    ).reshape(CP, CP).astype(bf16)
    j6 = np.einsum(
        "jm,k->jkm", np.eye(RG, dtype=np.float32), np.ones(C, np.float32)
    ).reshape(CP, RG).astype(bf16)
    ident = np.eye(P, dtype=np.float32).astype(bf16)

    in_maps = []
    offs = []
    for core in range(8):
        b, hc = core // 4, core % 4
        uuc, ewp, ys, off = _prep_core(u, ew, b, hc)
        offs.append(off)
        uuc16 = uuc.astype(bf16)
        # iteration-0 phase12 on the host: E = exp(-u), m = (LC/8) @ E,
        # r = 1 / sum_c E, rounded at the same points as the device path
        e16f = np.exp(-uuc16.astype(np.float32)).astype(bf16).astype(np.float32)
        lcf = (lc / 8.0).astype(bf16).astype(np.float32)
        m0 = np.einsum("lk,pxky->pxly", lcf, e16f).astype(bf16)
        r0 = (1.0 / e16f.sum(axis=2)).astype(bf16)

        # host-precomputed x+-1 shifts of m0/r0 (global x = xb*128 + p;
        # zero beyond the image edge), matching the device DMA shifts
        def shx(t, s):  # t: [P, XB, ...], shift source x+s
            f = np.moveaxis(t, 1, 0).reshape((XB * P,) + t.shape[2:])
            o = np.zeros_like(f)
            if s == 1:
                o[: XB * P - 1] = f[1:]
            else:
                o[1:] = f[: XB * P - 1]
            return np.moveaxis(o.reshape((XB, P) + t.shape[2:]), 0, 1)

        mxp0 = shx(m0, 1)
        mxm0 = shx(m0, -1)
        rp0 = shx(r0, 1)
        rm0 = shx(r0, -1)
        in_maps.append(
            {
                "uuc": uuc16,
                "mx0in": m0,
                "mxp0in": mxp0,
                "mxm0in": mxm0,
                "r0in": r0,
                "rp0in": rp0,
                "rm0in": rm0,
                "ew": ewp.astype(bf16),
                "lcblk": lcb,
                "j6": j6,
                "ident": ident,
            }
        )

    kwargs = {}
    if _trace:
        kwargs = dict(trace=True, trace_cores=[0], tmpdir=_tmpdir)
    res = run_bass_kernel_spmd(nc, in_maps, core_ids=list(range(8)), **kwargs)

    out = np.zeros((2, 1, C, 512, 512), dtype=np.float32)
    for core in range(8):
        b, hc = core // 4, core % 4
        yo = res.results[core]["yout"]            # [P, XB, C, YT] bf16
        slab = yo.astype(np.float32).transpose(2, 3, 1, 0).reshape(C, YT, W)
        off = offs[core]
        out[b, 0, :, 128 * hc : 128 * (hc + 1), :] = slab[:, off : off + OWN, :]
    if _trace:
        return out, res
    return out

